# revision 1
# baseline (speedup 1.0000x reference)
"""GNN message-passing (NodeModel) kernel for 8 Trainium2 NeuronCores.

Strategy (node-sharded, zero collectives):
  - Host: sort edges by destination node, deal nodes round-robin-by-degree
    across the 8 cores (near-perfect edge balance, and identical per-core
    degree histograms so one SPMD program serves all cores). Lay each core's
    edge rows out in (degree-bucket, column, partition, slot) order so the
    device consumes them with plain dense strided DMA loads - no gather.
  - Device, per core: for each exact-degree bucket, stream edge tiles into
    SBUF, reduce sum/max over the slot axis with strided TensorReduce
    (slots land in the free dimension of one partition), transpose the
    [node, feat] partials to [feat, node] on the PE, and assemble the MLP
    input h^T in SBUF (mean = sum * (1/deg) fused into the ScalarE copy,
    exact because buckets hold a single degree). Then a fused MLP:
    h2^T = relu(W1A^T @ hA + W1B^T @ hB + b1), out^T = W2^T @ h2 + x^T + b2
    with the residual added by an identity matmul accumulating into PSUM.
  - Host: scatter per-core outputs back to the original node order.
"""

import os
import numpy as np

P = 128          # SBUF partitions
FEAT = 48        # EDGE_OUT
XF = 64          # NODE_IN
HID = 128
NCORES = 8
MAXW = 512       # matmul moving free dim / PSUM bank in fp32

LAST_EXEC_NS = None
LAST_TRACE_DIR = None

_PROG_CACHE = {}


# ----------------------------------------------------------------- host plan

def _chunk_cols(d):
    return max(1, min(112 // d, 8)) if d <= P else 1



def _make_plan(deg, ncores):
    """Bucket nodes by exact degree; uniform per-core bucket sizes."""
    order = np.argsort(deg, kind="stable")          # nodes sorted by degree
    uniq, counts = np.unique(deg[order], return_counts=True)
    buckets = []                                     # (d, M_core)
    starts = np.concatenate([[0], np.cumsum(counts)])
    for d, cnt, s in zip(uniq.tolist(), counts.tolist(), starts[:-1].tolist()):
        m = -(-cnt // ncores)                        # ceil
        buckets.append((int(d), int(m), int(s), int(cnt)))
    # tiny buckets overlap the DMA-bound ramp at the start and form a short
    # final chain at the end; the dense big buckets sit in the middle where
    # the pipeline is saturated
    buckets.sort(key=lambda b: b[0] * b[1])
    small = [b for b in buckets if b[0] * b[1] * FEAT * 4 < 500_000]
    big = [b for b in buckets if b[0] * b[1] * FEAT * 4 >= 500_000]
    buckets = small[0::2] + big + small[1::2][::-1]
    nk = sum(m for _, m, _, _ in buckets)
    e_core = sum(d * m for d, m, _, _ in buckets)
    return {"order": order, "buckets": buckets, "nk": nk, "e_core": e_core}


def _host_pack(x, edge_index, edge_attr, u, batch, plan, ncores):
    N = x.shape[0]
    E = edge_attr.shape[0]
    col = np.asarray(edge_index[1], dtype=np.int64)
    deg = np.bincount(col, minlength=N)
    eperm = np.argsort(col, kind="stable")
    node_ptr = np.zeros(N + 1, np.int64)
    node_ptr[1:] = np.cumsum(deg)

    order = plan["order"]
    nk = plan["nk"]

    # per-core node permutation (sentinel N for padding) and edge id stream
    pis = np.full((ncores, nk), N, dtype=np.int64)
    eidx = [[] for _ in range(ncores)]
    coff = 0
    for d, m, s, cnt in plan["buckets"]:
        block = order[s:s + cnt]
        for k in range(ncores):
            mine = block[k::ncores]
            pis[k, coff:coff + len(mine)] = mine
            if d > 0:
                idx = np.full((m, d), E, dtype=np.int64)
                if len(mine):
                    starts = node_ptr[mine]
                    idx[:len(mine)] = starts[:, None] + np.arange(d)[None, :]
                    idx[:len(mine)] = eperm[idx[:len(mine)]]
                eidx[k].append(idx.ravel())
        coff += m

    ea = np.asarray(edge_attr, dtype=np.float32)
    ub = np.asarray(u, dtype=np.float32)[np.asarray(batch, dtype=np.int64), 0]

    ea_slices, xT, urow = [], [], []
    nzbuckets = [b for b in plan["buckets"] if b[0] > 0]
    for k in range(ncores):
        parts = []
        for arr, (d, m, s, cnt) in zip(eidx[k], nzbuckets):
            sent = arr == E
            fi_safe = np.where(sent, 0, arr)
            blk = ea[fi_safe]
            if sent.any():
                blk[sent] = 0.0
            # [m*d, 48] -> [m, 48, d]: feature-major per node so the device
            # reduce reads a contiguous innermost (slot) axis
            blk = np.ascontiguousarray(blk.reshape(m, d, FEAT).swapaxes(1, 2))
            # reorder nodes partition-major per device chunk so every edge
            # DMA is one contiguous run per partition (cheap HWDGE descgen)
            ktot = -(-m // P)
            last_r = m - (ktot - 1) * P
            ck = _chunk_cols(d)
            rows = []
            for c0 in range(0, ktot, ck):
                c1 = min(c0 + ck, ktot)
                kcf = (c1 - c0) if (c1 < ktot or last_r == P) else (c1 - c0 - 1)
                if kcf:
                    jm = (c0 + np.arange(kcf))[None, :] * P + np.arange(P)[:, None]
                    rows.append(jm.ravel())
                if kcf < c1 - c0:
                    rows.append((ktot - 1) * P + np.arange(last_r))
            parts.append(np.ascontiguousarray(
                blk[np.concatenate(rows)]).reshape(-1))
        if parts:
            flat = np.concatenate(parts)
            ea_slices.append(flat.reshape(-1, FEAT))
        else:
            ea_slices.append(np.zeros((1, FEAT), np.float32))

        pk = pis[k]
        sentn = pk == N
        pk_safe = np.where(sentn, 0, pk)
        xk = np.asarray(x, dtype=np.float32)[pk_safe]
        uk = ub[pk_safe].copy()
        if sentn.any():
            xk[sentn] = 0.0
            uk[sentn] = 0.0
        xT.append(np.ascontiguousarray(xk.T))
        urow.append(np.ascontiguousarray(uk[None, :]))

    return pis, ea_slices, xT, urow


# ------------------------------------------------------------- device program

def _build_program(buckets, nk, e_core, ncores):
    import concourse.bacc as bacc
    import concourse.mybir as mybir
    import concourse.tile as tile
    from concourse.masks import make_identity

    f32 = mybir.dt.float32
    nc = bacc.Bacc("TRN2", target_bir_lowering=False, debug=False,
                   num_devices=ncores)

    ea = nc.dram_tensor("ea", [max(e_core, 1), FEAT], f32, kind="ExternalInput")
    xT = nc.dram_tensor("xT", [XF, nk], f32, kind="ExternalInput")
    urow = nc.dram_tensor("urow", [1, nk], f32, kind="ExternalInput")
    w1a_d = nc.dram_tensor("W1A", [P, HID], f32, kind="ExternalInput")
    w1b_d = nc.dram_tensor("W1B", [XF + FEAT + 1, HID], f32, kind="ExternalInput")
    w2_d = nc.dram_tensor("W2", [HID, XF], f32, kind="ExternalInput")
    b1_d = nc.dram_tensor("b1", [HID, 1], f32, kind="ExternalInput")
    b2_d = nc.dram_tensor("b2", [XF, 1], f32, kind="ExternalInput")
    zpad_d = nc.dram_tensor("zpad", [16, nk], f32, kind="ExternalInput")
    outT = nc.dram_tensor("outT", [XF, nk], f32, kind="ExternalOutput")

    NB = XF + FEAT + 1  # 113 rows in hB: m(0:48) zeros(48:64) mean(64:112) u(112)

    with tile.TileContext(nc) as tc:
        with tc.tile_pool(name="const", bufs=1) as cp, \
             tc.tile_pool(name="hp", bufs=1) as hp, \
             tc.tile_pool(name="edges", bufs=3) as ep, \
             tc.tile_pool(name="red", bufs=3) as rp, \
             tc.tile_pool(name="mlp", bufs=2) as mp, \
             tc.tile_pool(name="pst", bufs=4, space="PSUM") as pst, \
             tc.tile_pool(name="psm", bufs=2, space="PSUM") as psm:

            ident = cp.tile([P, P], f32)
            make_identity(nc, ident[:])
            w1a = cp.tile([P, HID], f32)
            nc.scalar.dma_start(out=w1a[:], in_=w1a_d[:, :])
            w1b = cp.tile([NB, HID], f32)
            nc.scalar.dma_start(out=w1b[:], in_=w1b_d[:, :])
            w2 = cp.tile([HID, XF], f32)
            nc.scalar.dma_start(out=w2[:], in_=w2_d[:, :])
            b1t = cp.tile([HID, 1], f32)
            nc.scalar.dma_start(out=b1t[:], in_=b1_d[:, :])
            b2t = cp.tile([XF, 1], f32)
            nc.scalar.dma_start(out=b2t[:], in_=b2_d[:, :])

            hA = hp.tile([XF + FEAT, nk], f32)  # x(64) | s(48)
            hB = hp.tile([NB, nk], f32)        # m(48) | 0(16) | mean(48) | u
            nc.scalar.dma_start(out=hB[NB - 1:NB, :], in_=urow[:, :])
            # hB rows 48:64 must be zero (inside the mm1b contraction); x^T
            # streams just-in-time per MLP chunk pair; hA has no pad rows
            nc.scalar.dma_start(out=hB[FEAT:XF, :], in_=zpad_d[:, :])
            # prime every red-pool slot with zeros: the [48:64] per-column gap
            # in the combined reduce layout is then always finite (its value
            # is never consumed - it only streams through the PE transpose)
            for _ in range(3):
                tmp = rp.tile([P, 8 * (2 * FEAT + 16)], f32, tag="s")
                nc.scalar.memzero(tmp[:])

            # ---- fused node MLP, emitted per column chunk as soon as the
            # producing buckets have been emitted (lets Tile overlap the MLP
            # with later buckets' DMA/reduce work)
            def emit_mlp_chunks(qs):
                # pair-emit so matmuls sharing the same stationary weights run
                # back-to-back on the PE (cheaper weight reloads)
                ws = [min(MAXW, nk - q) for q in qs]
                lo = qs[0]
                hi = qs[-1] + min(MAXW, nk - qs[-1])
                nc.scalar.dma_start(out=hA[0:XF, lo:hi], in_=xT[:, lo:hi])
                pm1s, h2s, pm2s, ots = [], [], [], []
                for q0, w in zip(qs, ws):
                    pm1 = psm.tile([HID, MAXW], f32, tag="mm1")
                    nc.tensor.matmul(out=pm1[:, 0:w], lhsT=w1a[0:XF + FEAT, :],
                                     rhs=hA[0:XF + FEAT, q0:q0 + w],
                                     start=True, stop=False)
                    pm1s.append(pm1)
                for q0, w, pm1 in zip(qs, ws, pm1s):
                    nc.tensor.matmul(out=pm1[:, 0:w], lhsT=w1b[:],
                                     rhs=hB[:, q0:q0 + w], start=False, stop=True)
                for q0, w, pm1 in zip(qs, ws, pm1s):
                    h2 = mp.tile([HID, MAXW], f32, tag="h2")
                    nc.scalar.activation(out=h2[:, 0:w], in_=pm1[:, 0:w],
                                         func=mybir.ActivationFunctionType.Relu,
                                         bias=b1t[:, 0:1])
                    h2s.append(h2)
                for q0, w, h2 in zip(qs, ws, h2s):
                    pm2 = psm.tile([XF, MAXW], f32, tag="mm2")
                    nc.tensor.matmul(out=pm2[:, 0:w], lhsT=w2[:],
                                     rhs=h2[:, 0:w], start=True, stop=False)
                    pm2s.append(pm2)
                for q0, w, pm2 in zip(qs, ws, pm2s):
                    nc.tensor.matmul(out=pm2[:, 0:w], lhsT=ident[0:XF, 0:XF],
                                     rhs=hA[0:XF, q0:q0 + w],
                                     start=False, stop=True)
                for q0, w, pm2 in zip(qs, ws, pm2s):
                    ot = mp.tile([XF, MAXW], f32, tag="ot")
                    nc.scalar.activation(out=ot[:, 0:w], in_=pm2[:, 0:w],
                                         func=mybir.ActivationFunctionType.Identity,
                                         bias=b2t[:, 0:1])
                    ots.append(ot)
                for q0, w, ot in zip(qs, ws, ots):
                    nc.scalar.dma_start(out=outT[:, q0:q0 + w], in_=ot[:, 0:w])

            def emit_mlp_chunk(q0):
                emit_mlp_chunks([q0])

            col_off = 0
            e_off = 0
            mlp_q0 = 0
            for d, m, _, _ in buckets:
                if d == 0:
                    nc.vector.memset(hA[XF:XF + FEAT, col_off:col_off + m], 0.0)
                    nc.vector.memset(hB[0:NB - 1, col_off:col_off + m], 0.0)  # noqa: start 0 span 112
                    col_off += m
                    while mlp_q0 + MAXW <= col_off:
                        emit_mlp_chunk(mlp_q0)
                        mlp_q0 += MAXW
                    continue
                ktot = -(-m // P)               # total columns in bucket
                ck_cols = _chunk_cols(d)
                for c0 in range(0, ktot, ck_cols):
                    c1 = min(c0 + ck_cols, ktot)
                    kc = c1 - c0
                    # nodes covered: cols c0..c1-1; col j has 128 nodes except
                    # possibly the bucket's last col.
                    last_r = m - (ktot - 1) * P  # nodes in last col (1..128)
                    full_cols = kc if (c1 < ktot or last_r == P) else kc - 1
                    et = ep.tile([P, ck_cols * d * FEAT], f32, tag="e")
                    base = e_off + c0 * P * d
                    if full_cols:
                        src = ea[base:base + full_cols * P * d, :].rearrange(
                            "(p a) f -> p (a f)", p=P, a=full_cols * d)
                        nc.sync.dma_start(
                            out=et[:, 0:full_cols * d * FEAT], in_=src)
                    if full_cols < kc:
                        r = last_r
                        tb = base + full_cols * P * d
                        src = ea[tb:tb + r * d, :].rearrange(
                            "(p a) f -> p (a f)", p=r, a=d)
                        nc.sync.dma_start(
                            out=et[0:r, full_cols * d * FEAT:kc * d * FEAT],
                            in_=src)
                    # reduce slots (axis a) -> [128, kc, 48]; split into the
                    # full-column block and the ragged tail column so no
                    # uninitialized partitions are read.
                    # combined per-column [s(48) | pad(16) | m(48)] layout so
                    # one PE transpose moves both tensors; the pad lands at
                    # psum rows 48:64 which are never copied out.
                    SEG = 2 * FEAT + 16
                    st = rp.tile([P, 8 * SEG], f32, tag="s")
                    stv = st[:, 0:kc * SEG].rearrange("p (k c) -> p k c", k=kc)
                    if full_cols:
                        ev = et[:, 0:full_cols * d * FEAT].rearrange(
                            "p (k f a) -> p k f a", k=full_cols, a=d, f=FEAT)
                        nc.vector.reduce_sum(
                            out=stv[:, 0:full_cols, 0:FEAT], in_=ev,
                            axis=mybir.AxisListType.X)
                        nc.vector.reduce_max(
                            out=stv[:, 0:full_cols, FEAT + 16:SEG], in_=ev,
                            axis=mybir.AxisListType.X)
                    if full_cols < kc:
                        r = last_r
                        ev = et[0:r, full_cols * d * FEAT:kc * d * FEAT].rearrange(
                            "p (f a) -> p f a", a=d, f=FEAT)
                        nc.vector.reduce_sum(
                            out=stv[0:r, full_cols, 0:FEAT],
                            in_=ev, axis=mybir.AxisListType.X)
                        nc.vector.reduce_max(
                            out=stv[0:r, full_cols, FEAT + 16:SEG],
                            in_=ev, axis=mybir.AxisListType.X)
                    # transpose groups of up to 4 cols -> [112, <=512] PSUM
                    for g0 in range(0, kc, 4):
                        g1 = min(g0 + 4, kc)
                        ps_s = pst.tile([SEG, MAXW], f32, tag="ts")
                        cov = 0
                        for j in range(g0, g1):
                            pc = P if (c0 + j) < ktot - 1 or last_r == P else last_r
                            nc.tensor.transpose(
                                out=ps_s[:, (j - g0) * P:(j - g0) * P + pc],
                                in_=st[0:pc, j * SEG:(j + 1) * SEG],
                                identity=ident[0:pc, 0:pc])
                            cov = (j - g0) * P + pc
                        dst0 = col_off + (c0 + g0) * P
                        nc.scalar.copy(out=hA[XF:XF + FEAT, dst0:dst0 + cov],
                                       in_=ps_s[0:FEAT, 0:cov])
                        nc.scalar.mul(out=hB[XF:XF + FEAT, dst0:dst0 + cov],
                                      in_=ps_s[0:FEAT, 0:cov], mul=1.0 / d)
                        nc.scalar.copy(out=hB[0:FEAT, dst0:dst0 + cov],
                                       in_=ps_s[XF:XF + FEAT, 0:cov])
                col_off += m
                e_off += m * d
                ready = []
                while mlp_q0 + MAXW <= col_off:
                    ready.append(mlp_q0)
                    mlp_q0 += MAXW
                for i in range(0, len(ready), 2):
                    emit_mlp_chunks(ready[i:i + 2])

            while mlp_q0 < nk:
                emit_mlp_chunk(mlp_q0)
                mlp_q0 += MAXW

    nc.compile()
    return nc


# ----------------------------------------------------------------------- main

def kernel(**inputs):
    global LAST_EXEC_NS, LAST_TRACE_DIR
    from concourse.bass_utils import run_bass_kernel_spmd

    x = np.asarray(inputs["x"], dtype=np.float32)
    edge_index = np.asarray(inputs["edge_index"])
    edge_attr = np.asarray(inputs["edge_attr"], dtype=np.float32)
    u = np.asarray(inputs["u"], dtype=np.float32)
    batch = np.asarray(inputs["batch"])
    W1 = np.asarray(inputs["W1"], dtype=np.float32)
    b1 = np.asarray(inputs["b1"], dtype=np.float32)
    W2 = np.asarray(inputs["W2"], dtype=np.float32)
    b2 = np.asarray(inputs["b2"], dtype=np.float32)

    N = x.shape[0]
    col = np.asarray(edge_index[1], dtype=np.int64)
    deg = np.bincount(col, minlength=N)
    plan = _make_plan(deg, NCORES)
    buckets = plan["buckets"]
    nk, e_core = plan["nk"], plan["e_core"]

    key = (N, edge_attr.shape[0], tuple((d, m) for d, m, _, _ in buckets))
    if key not in _PROG_CACHE:
        _PROG_CACHE[key] = _build_program(buckets, nk, e_core, NCORES)
    nc = _PROG_CACHE[key]

    pis, ea_s, xT_s, u_s = _host_pack(x, edge_index, edge_attr, u, batch,
                                      plan, NCORES)

    zpad = np.zeros((16, nk), np.float32)
    mlp_in = W1.shape[0]                     # 209
    w1a = np.zeros((P, HID), np.float32)
    w1a[:XF + FEAT] = W1[:XF + FEAT]         # x + s rows
    w1b = np.zeros((XF + FEAT + 1, HID), np.float32)
    w1b[0:FEAT] = W1[XF + FEAT:XF + 2 * FEAT]          # m rows
    w1b[XF:XF + FEAT] = W1[XF + 2 * FEAT:XF + 3 * FEAT]  # mean rows
    w1b[XF + FEAT] = W1[mlp_in - 1]                    # u row
    in_maps = []
    for k in range(NCORES):
        in_maps.append({
            "ea": ea_s[k], "xT": xT_s[k], "urow": u_s[k],
            "W1A": w1a, "W1B": w1b,
            "W2": np.ascontiguousarray(W2),
            "zpad": zpad,
            "b1": np.ascontiguousarray(b1.reshape(HID, 1)),
            "b2": np.ascontiguousarray(b2.reshape(XF, 1)),
        })

    trace = bool(int(os.environ.get("KERNEL_TRACE", "0")))
    kwargs = {}
    if trace:
        tdir = os.environ.get("KERNEL_TRACE_DIR") or None
        kwargs = {"trace": True, "tmpdir": tdir}
    res = run_bass_kernel_spmd(nc, in_maps, core_ids=list(range(NCORES)),
                               **kwargs)
    LAST_EXEC_NS = res.exec_time_ns

    out = np.empty((N, XF), np.float32)
    for k in range(NCORES):
        ok = res.results[k]["outT"].T        # [nk, 64]
        pk = pis[k]
        valid = pk != N
        out[pk[valid]] = ok[valid]
    return out



# revision 5
# speedup vs baseline: 1.2130x; 1.2130x over previous
"""GNN message-passing (NodeModel) kernel for 8 Trainium2 NeuronCores.

Strategy (node-sharded, zero collectives, bf16 data path):
  - Host: sort edges by destination node, deal nodes round-robin-by-degree
    across the 8 cores (near-perfect edge balance and identical per-core
    degree histograms, so one SPMD program serves all cores). For each
    exact-degree bucket the per-core node count is padded to a multiple of
    128 (pad nodes get zero edges), so every device chunk is a dense
    [128, kc*d*48] tile with uniform strides - no ragged tails anywhere.
    Edge data is laid out slot-major (p, slot, col, feat) in bf16.
  - Device, per chunk: one DMA, then pairwise tensor_tensor reduction
    trees on the Vector engine (bf16 tensor_tensor runs in 2x mode - twice
    the throughput of tensor_reduce, which is capped at 1x regardless of
    dtype): sum-tree with odd-carry copies (carries on the Scalar engine),
    max-tree with overlapped folds (idempotent, no carries). Tree leaves
    land in [128, kc*48] tiles; PE transposes per 128-node column put
    s / m into a [96, 512] PSUM tile, and ScalarE copies them into the MLP
    input (mean = s * (1/d) fused, exact because buckets hold one degree).
  - Fused node MLP in bf16 (PSUM accumulates fp32):
    h2 = relu(W1a^T @ [x; s] + W1b^T @ [m; mean; u] + b1)
    out^T = W2^T @ h2 + x^T + b2, residual added via identity matmul.
  - Host: scatter per-core outputs back to the original node order.
"""

import os
import numpy as np
import ml_dtypes

P = 128          # SBUF partitions
FEAT = 48        # EDGE_OUT
XF = 64          # NODE_IN
HID = 128
NB = XF + FEAT + 1  # 113 rows in hB: m(0:48) 0(48:64) mean(64:112) u(112)
NCORES = 8
MAXW = 512       # matmul moving free dim / PSUM bank in fp32

BF16 = ml_dtypes.bfloat16

LAST_EXEC_NS = None

_PROG_CACHE = {}


# ----------------------------------------------------------------- host plan

def _chunk_cols(d):
    return max(1, min(224 // d, 12))


def _make_plan(deg, ncores):
    """Bucket nodes by exact degree; uniform per-core bucket sizes."""
    order = np.argsort(deg, kind="stable")          # nodes sorted by degree
    uniq, counts = np.unique(deg[order], return_counts=True)
    buckets = []                                     # (d, m_core, start, cnt)
    starts = np.concatenate([[0], np.cumsum(counts)])
    for d, cnt, s in zip(uniq.tolist(), counts.tolist(), starts[:-1].tolist()):
        m = -(-cnt // ncores)                        # ceil
        buckets.append((int(d), int(m), int(s), int(cnt)))
    # tiny buckets overlap the DMA-bound ramp at the start and form a short
    # final chain at the end; the dense big buckets sit in the middle where
    # the pipeline is saturated
    buckets.sort(key=lambda b: b[0] * b[1])
    small = [b for b in buckets if b[0] * b[1] * FEAT * 2 < 250_000]
    big = [b for b in buckets if b[0] * b[1] * FEAT * 2 >= 250_000]
    buckets = small[0::2] + big + small[1::2][::-1]
    nk = sum(m for _, m, _, _ in buckets)
    # padded edge-stream element count (bf16 elems)
    e_total = 0
    for d, m, _, _ in buckets:
        if d > 0:
            e_total += -(-m // P) * P * d * FEAT
    return {"order": order, "buckets": buckets, "nk": nk, "e_total": e_total}


def _host_pack(x, edge_index, edge_attr, u, batch, plan, ncores):
    N = x.shape[0]
    E = edge_attr.shape[0]
    col = np.asarray(edge_index[1], dtype=np.int64)
    deg = np.bincount(col, minlength=N)
    eperm = np.argsort(col, kind="stable")
    node_ptr = np.zeros(N + 1, np.int64)
    node_ptr[1:] = np.cumsum(deg)

    order = plan["order"]
    nk = plan["nk"]

    ea = np.asarray(edge_attr, dtype=np.float32)
    ub = np.asarray(u, dtype=np.float32)[np.asarray(batch, dtype=np.int64), 0]

    # per-core node permutation (sentinel N for padding)
    pis = np.full((ncores, nk), N, dtype=np.int64)
    coff = 0
    for d, m, s, cnt in plan["buckets"]:
        block = order[s:s + cnt]
        for k in range(ncores):
            mine = block[k::ncores]
            pis[k, coff:coff + len(mine)] = mine
        coff += m

    ea_streams, xT, urow = [], [], []
    for k in range(ncores):
        parts = []
        for d, m, s, cnt in plan["buckets"]:
            if d == 0:
                continue
            mine = order[s:s + cnt][k::ncores]
            ktot = -(-m // P)
            M = ktot * P
            idx = np.full((M, d), E, dtype=np.int64)
            if len(mine):
                idx[:len(mine)] = node_ptr[mine][:, None] + np.arange(d)[None, :]
                idx[:len(mine)] = eperm[idx[:len(mine)]]
            sent = idx == E
            blk = ea[np.where(sent, 0, idx)]
            if sent.any():
                blk[sent] = 0.0
            blk = blk.reshape(M, d, FEAT)
            ck = _chunk_cols(d)
            for c0 in range(0, ktot, ck):
                kc = min(ck, ktot - c0)
                sub = blk[c0 * P:(c0 + kc) * P]          # [kc*128, d, 48]
                sub = sub.reshape(kc, P, d, FEAT).transpose(1, 2, 0, 3)
                parts.append(np.ascontiguousarray(sub).ravel())
        if parts:
            flat = np.concatenate(parts).astype(BF16)
        else:
            flat = np.zeros(P, np.float32).astype(BF16)
        ea_streams.append(flat)

        pk = pis[k]
        sentn = pk == N
        pk_safe = np.where(sentn, 0, pk)
        xk = np.asarray(x, dtype=np.float32)[pk_safe]
        uk = ub[pk_safe].copy()
        if sentn.any():
            xk[sentn] = 0.0
            uk[sentn] = 0.0
        xT.append(np.ascontiguousarray(xk.T).astype(BF16))
        urow.append(np.ascontiguousarray(uk[None, :]).astype(BF16))

    return pis, ea_streams, xT, urow


# ------------------------------------------------------------- device program

def _build_program(buckets, nk, e_total, ncores):
    import concourse.bacc as bacc
    import concourse.mybir as mybir
    import concourse.tile as tile
    from concourse.masks import make_identity

    f32 = mybir.dt.float32
    bf16 = mybir.dt.bfloat16
    nc = bacc.Bacc("TRN2", target_bir_lowering=False, debug=False,
                   num_devices=ncores)

    ea = nc.dram_tensor("ea", [max(e_total, P)], bf16, kind="ExternalInput")
    xT = nc.dram_tensor("xT", [XF, nk], bf16, kind="ExternalInput")
    urow = nc.dram_tensor("urow", [1, nk], bf16, kind="ExternalInput")
    w1a_d = nc.dram_tensor("W1A", [XF + FEAT, HID], bf16, kind="ExternalInput")
    w1b_d = nc.dram_tensor("W1B", [NB, HID], bf16, kind="ExternalInput")
    w2_d = nc.dram_tensor("W2", [HID, XF], bf16, kind="ExternalInput")
    b1_d = nc.dram_tensor("b1", [HID, 1], f32, kind="ExternalInput")
    b2_d = nc.dram_tensor("b2", [XF, 1], f32, kind="ExternalInput")
    outT = nc.dram_tensor("outT", [XF, nk], bf16, kind="ExternalOutput")

    with tile.TileContext(nc) as tc:
        with tc.tile_pool(name="const", bufs=1) as cp, \
             tc.tile_pool(name="hp", bufs=1) as hp, \
             tc.tile_pool(name="edges", bufs=3) as ep, \
             tc.tile_pool(name="tree", bufs=1) as tp, \
             tc.tile_pool(name="red", bufs=3) as rp, \
             tc.tile_pool(name="mlp", bufs=2) as mp, \
             tc.tile_pool(name="pst", bufs=4, space="PSUM") as pst, \
             tc.tile_pool(name="psm", bufs=2, space="PSUM") as psm:

            ident = cp.tile([P, P], bf16)
            make_identity(nc, ident[:])
            w1a = cp.tile([XF + FEAT, HID], bf16)
            nc.scalar.dma_start(out=w1a[:], in_=w1a_d[:, :])
            w1b = cp.tile([NB, HID], bf16)
            nc.scalar.dma_start(out=w1b[:], in_=w1b_d[:, :])
            w2 = cp.tile([HID, XF], bf16)
            nc.scalar.dma_start(out=w2[:], in_=w2_d[:, :])
            b1t = cp.tile([HID, 1], f32)
            nc.scalar.dma_start(out=b1t[:], in_=b1_d[:, :])
            b2t = cp.tile([XF, 1], f32)
            nc.scalar.dma_start(out=b2t[:], in_=b2_d[:, :])

            hA = hp.tile([XF + FEAT, nk], bf16)  # x(0:64) | s(64:112)
            hB = hp.tile([NB, nk], bf16)  # m(0:48) | 0(48:64) | mean(64:112) | u
            nc.vector.memset(hB[:, :], 0.0)
            nc.scalar.dma_start(out=hB[NB - 1:NB, :], in_=urow[:, :])

            # ---- fused node MLP, emitted per column chunk as soon as the
            # producing buckets have been emitted (lets Tile overlap the MLP
            # with later buckets' DMA/reduce work)
            def emit_mlp_chunks(qs):
                ws = [min(MAXW, nk - q) for q in qs]
                lo = qs[0]
                hi = qs[-1] + min(MAXW, nk - qs[-1])
                nc.scalar.dma_start(out=hA[0:XF, lo:hi], in_=xT[:, lo:hi])
                pm1s, h2s, pm2s, ots = [], [], [], []
                for q0, w in zip(qs, ws):
                    pm1 = psm.tile([HID, MAXW], f32, tag="mm1")
                    nc.tensor.matmul(out=pm1[:, 0:w], lhsT=w1a[:],
                                     rhs=hA[:, q0:q0 + w],
                                     start=True, stop=False)
                    pm1s.append(pm1)
                for q0, w, pm1 in zip(qs, ws, pm1s):
                    nc.tensor.matmul(out=pm1[:, 0:w], lhsT=w1b[:],
                                     rhs=hB[:, q0:q0 + w], start=False, stop=True)
                for q0, w, pm1 in zip(qs, ws, pm1s):
                    h2 = mp.tile([HID, MAXW], bf16, tag="h2")
                    nc.scalar.activation(out=h2[:, 0:w], in_=pm1[:, 0:w],
                                         func=mybir.ActivationFunctionType.Relu,
                                         bias=b1t[:, 0:1])
                    h2s.append(h2)
                for q0, w, h2 in zip(qs, ws, h2s):
                    pm2 = psm.tile([XF, MAXW], f32, tag="mm2")
                    nc.tensor.matmul(out=pm2[:, 0:w], lhsT=w2[:],
                                     rhs=h2[:, 0:w], start=True, stop=False)
                    pm2s.append(pm2)
                for q0, w, pm2 in zip(qs, ws, pm2s):
                    nc.tensor.matmul(out=pm2[:, 0:w], lhsT=ident[0:XF, 0:XF],
                                     rhs=hA[0:XF, q0:q0 + w],
                                     start=False, stop=True)
                for q0, w, pm2 in zip(qs, ws, pm2s):
                    ot = mp.tile([XF, MAXW], bf16, tag="ot")
                    nc.scalar.activation(out=ot[:, 0:w], in_=pm2[:, 0:w],
                                         func=mybir.ActivationFunctionType.Identity,
                                         bias=b2t[:, 0:1])
                    ots.append(ot)
                for q0, w, ot in zip(qs, ws, ots):
                    nc.scalar.dma_start(out=outT[:, q0:q0 + w], in_=ot[:, 0:w])

            def emit_mlp_chunk(q0):
                emit_mlp_chunks([q0])

            # ---- pairwise reduction trees (Vector engine, bf16 2x mode)
            def emit_sum_tree(et, d, seg, stS):
                n, cur = d, et
                cA = cB = None
                use_a = True
                while n > 1:
                    h = n // 2
                    odd = n % 2
                    if h == 1 and not odd:
                        dst = stS
                    elif use_a:
                        if cA is None:
                            cA = tp.tile([P, (d // 2 + 1) * seg], bf16, tag="sA")
                        dst = cA
                    else:
                        if cB is None:
                            cB = tp.tile([P, (d // 4 + 2) * seg], bf16, tag="sB")
                        dst = cB
                    nc.vector.tensor_add(out=dst[:, 0:h * seg],
                                         in0=cur[:, 0:h * seg],
                                         in1=cur[:, h * seg:2 * h * seg])
                    if odd:
                        nc.scalar.copy(out=dst[:, h * seg:(h + 1) * seg],
                                       in_=cur[:, 2 * h * seg:(2 * h + 1) * seg])
                    n = h + odd
                    cur = dst
                    use_a = not use_a

            def emit_max_tree(et, d, seg, stM):
                n, cur = d, et
                cA = cB = None
                use_a = True
                while n > 1:
                    h = (n + 1) // 2
                    if h == 1:
                        dst = stM
                    elif use_a:
                        if cA is None:
                            cA = tp.tile([P, ((d + 1) // 2) * seg], bf16, tag="mA")
                        dst = cA
                    else:
                        if cB is None:
                            cB = tp.tile([P, ((d + 3) // 4) * seg], bf16, tag="mB")
                        dst = cB
                    nc.vector.tensor_max(out=dst[:, 0:h * seg],
                                         in0=cur[:, 0:h * seg],
                                         in1=cur[:, (n - h) * seg:n * seg])
                    n = h
                    cur = dst
                    use_a = not use_a

            col_off = 0
            e_off = 0
            mlp_q0 = 0
            for d, m, _, _ in buckets:
                if d == 0:
                    nc.vector.memset(hA[XF:XF + FEAT, col_off:col_off + m], 0.0)
                    nc.vector.memset(hB[0:NB - 1, col_off:col_off + m], 0.0)
                    col_off += m
                    while mlp_q0 + MAXW <= col_off:
                        emit_mlp_chunk(mlp_q0)
                        mlp_q0 += MAXW
                    continue
                ktot = -(-m // P)
                ck = _chunk_cols(d)
                for c0 in range(0, ktot, ck):
                    kc = min(ck, ktot - c0)
                    seg = kc * FEAT
                    sz = P * d * seg
                    et = ep.tile([P, ck * d * FEAT], bf16, tag="e")
                    nc.sync.dma_start(
                        out=et[:, 0:d * seg],
                        in_=ea[e_off:e_off + sz].rearrange("(p x) -> p x", p=P))
                    e_off += sz
                    if d == 1:
                        stS = et
                        stM = et
                    else:
                        stS = rp.tile([P, ck * FEAT], bf16, tag="ss")
                        stM = rp.tile([P, ck * FEAT], bf16, tag="sm")
                        emit_sum_tree(et, d, seg, stS)
                        emit_max_tree(et, d, seg, stM)
                    # transpose to [48, node] and assemble MLP input rows
                    for g0 in range(0, kc, 4):
                        g1 = min(g0 + 4, kc)
                        valid = m - (c0 + g0) * P
                        if valid <= 0:
                            break
                        ps = pst.tile([XF + FEAT, MAXW], bf16, tag="ts")
                        for j in range(g0, g1):
                            if m - (c0 + j) * P <= 0:
                                break
                            o = (j - g0) * P
                            nc.tensor.transpose(
                                out=ps[0:FEAT, o:o + P],
                                in_=stS[:, j * FEAT:(j + 1) * FEAT],
                                identity=ident[:, :])
                            nc.tensor.transpose(
                                out=ps[XF:XF + FEAT, o:o + P],
                                in_=stM[:, j * FEAT:(j + 1) * FEAT],
                                identity=ident[:, :])
                        cov = min((g1 - g0) * P, valid)
                        dst0 = col_off + (c0 + g0) * P
                        nc.scalar.copy(out=hA[XF:XF + FEAT, dst0:dst0 + cov],
                                       in_=ps[0:FEAT, 0:cov])
                        nc.scalar.copy(out=hB[0:FEAT, dst0:dst0 + cov],
                                       in_=ps[XF:XF + FEAT, 0:cov])
                        nc.scalar.mul(out=hB[XF:XF + FEAT, dst0:dst0 + cov],
                                      in_=ps[0:FEAT, 0:cov], mul=1.0 / d)
                col_off += m
                ready = []
                while mlp_q0 + MAXW <= col_off:
                    ready.append(mlp_q0)
                    mlp_q0 += MAXW
                for i in range(0, len(ready), 2):
                    emit_mlp_chunks(ready[i:i + 2])

            while mlp_q0 < nk:
                emit_mlp_chunk(mlp_q0)
                mlp_q0 += MAXW

    nc.compile()
    return nc


# ----------------------------------------------------------------------- main

def kernel(**inputs):
    global LAST_EXEC_NS
    from concourse.bass_utils import run_bass_kernel_spmd

    x = np.asarray(inputs["x"], dtype=np.float32)
    edge_index = np.asarray(inputs["edge_index"])
    edge_attr = np.asarray(inputs["edge_attr"], dtype=np.float32)
    u = np.asarray(inputs["u"], dtype=np.float32)
    batch = np.asarray(inputs["batch"])
    W1 = np.asarray(inputs["W1"], dtype=np.float32)
    b1 = np.asarray(inputs["b1"], dtype=np.float32)
    W2 = np.asarray(inputs["W2"], dtype=np.float32)
    b2 = np.asarray(inputs["b2"], dtype=np.float32)

    N = x.shape[0]
    col = np.asarray(edge_index[1], dtype=np.int64)
    deg = np.bincount(col, minlength=N)
    plan = _make_plan(deg, NCORES)
    buckets = plan["buckets"]
    nk, e_total = plan["nk"], plan["e_total"]

    key = (N, edge_attr.shape[0], tuple((d, m) for d, m, _, _ in buckets))
    if key not in _PROG_CACHE:
        _PROG_CACHE[key] = _build_program(buckets, nk, e_total, NCORES)
    nc = _PROG_CACHE[key]

    pis, ea_s, xT_s, u_s = _host_pack(x, edge_index, edge_attr, u, batch,
                                      plan, NCORES)

    mlp_in = W1.shape[0]                     # 209
    w1a = W1[0:XF + FEAT].astype(BF16)                 # x + s rows
    w1b = np.zeros((NB, HID), np.float32)
    w1b[0:FEAT] = W1[XF + FEAT:XF + 2 * FEAT]          # m rows
    w1b[XF:XF + FEAT] = W1[XF + 2 * FEAT:XF + 3 * FEAT]  # mean rows
    w1b[XF + FEAT] = W1[mlp_in - 1]                    # u row
    in_maps = []
    for k in range(NCORES):
        in_maps.append({
            "ea": ea_s[k], "xT": xT_s[k], "urow": u_s[k],
            "W1A": np.ascontiguousarray(w1a),
            "W1B": w1b.astype(BF16),
            "W2": np.ascontiguousarray(W2).astype(BF16),
            "b1": np.ascontiguousarray(b1.reshape(HID, 1)),
            "b2": np.ascontiguousarray(b2.reshape(XF, 1)),
        })

    trace = bool(int(os.environ.get("KERNEL_TRACE", "0")))
    kwargs = {}
    if trace:
        tdir = os.environ.get("KERNEL_TRACE_DIR") or None
        kwargs = {"trace": True, "tmpdir": tdir}
    res = run_bass_kernel_spmd(nc, in_maps, core_ids=list(range(NCORES)),
                               **kwargs)
    LAST_EXEC_NS = res.exec_time_ns

    out = np.empty((N, XF), np.float32)
    for k in range(NCORES):
        ok = res.results[k]["outT"].T.astype(np.float32)   # [nk, 64]
        pk = pis[k]
        valid = pk != N
        out[pk[valid]] = ok[valid]
    return out


# revision 15
# speedup vs baseline: 1.3929x; 1.1483x over previous
"""GNN message-passing (NodeModel) kernel for 8 Trainium2 NeuronCores.

Strategy (node-sharded, zero collectives, bf16 data path):
  - Host: sort edges by destination node, deal nodes round-robin-by-degree
    across the 8 cores (near-perfect edge balance and identical per-core
    degree histograms, so one SPMD program serves all cores). For each
    exact-degree bucket the per-core node count is padded to a multiple of
    128 (pad nodes get zero edges), so every device chunk is a dense
    [128, kc*d*48] tile with uniform strides - no ragged tails anywhere.
    Edge data is laid out slot-major (p, slot, col, feat) in bf16.
  - Device, per chunk: one DMA, then pairwise tensor_tensor reduction
    trees on the Vector engine (bf16 tensor_tensor runs in 2x mode - twice
    the throughput of tensor_reduce, which is capped at 1x regardless of
    dtype): sum-tree with odd-carry copies (carries on the Scalar engine),
    max-tree with overlapped folds (idempotent, no carries). Tree leaves
    land in [128, kc*48] tiles; PE transposes per 128-node column put
    s / m into a [96, 512] PSUM tile, and ScalarE copies them into the MLP
    input (mean = s * (1/d) fused, exact because buckets hold one degree).
  - Fused node MLP in bf16 (PSUM accumulates fp32):
    h2 = relu(W1a^T @ [x; s] + W1b^T @ [m; mean; u] + b1)
    out^T = W2^T @ h2 + x^T + b2, residual added via identity matmul.
  - Host: scatter per-core outputs back to the original node order.
"""

import os
import numpy as np
import ml_dtypes

P = 128          # SBUF partitions
FEAT = 48        # EDGE_OUT
XF = 64          # NODE_IN
HID = 128
NB = XF + FEAT + 1  # 113 rows in hB: m(0:48) 0(48:64) mean(64:112) u(112)
NCORES = 8
MAXW = 512       # matmul moving free dim / PSUM bank in fp32

BF16 = ml_dtypes.bfloat16

LAST_EXEC_NS = None

_PROG_CACHE = {}


# ----------------------------------------------------------------- host plan

def _chunk_cols(d):
    return max(1, min(224 // d, 16))


def _make_plan(deg, ncores):
    """Bucket nodes by exact degree; uniform per-core bucket sizes."""
    order = np.argsort(deg, kind="stable")          # nodes sorted by degree
    uniq, counts = np.unique(deg[order], return_counts=True)
    buckets = []                                     # (d, m_core, start, cnt)
    starts = np.concatenate([[0], np.cumsum(counts)])
    for d, cnt, s in zip(uniq.tolist(), counts.tolist(), starts[:-1].tolist()):
        m = -(-cnt // ncores)                        # ceil
        buckets.append((int(d), int(m), int(s), int(cnt)))
    # tiny buckets overlap the DMA-bound ramp at the start and form a short
    # final chain at the end; the dense big buckets sit in the middle where
    # the pipeline is saturated
    buckets.sort(key=lambda b: b[0] * b[1])
    small = [b for b in buckets if b[0] * b[1] * FEAT * 2 < 250_000]
    big = [b for b in buckets if b[0] * b[1] * FEAT * 2 >= 250_000]
    buckets = small[0::2] + big + small[1::2][::-1]
    nk = sum(m for _, m, _, _ in buckets)
    # padded edge-stream element count (bf16 elems)
    e_total = 0
    for d, m, _, _ in buckets:
        if d > 0:
            e_total += -(-m // P) * P * d * FEAT
    return {"order": order, "buckets": buckets, "nk": nk, "e_total": e_total}


def _host_pack(x, edge_index, edge_attr, u, batch, plan, ncores):
    N = x.shape[0]
    E = edge_attr.shape[0]
    col = np.asarray(edge_index[1], dtype=np.int64)
    deg = np.bincount(col, minlength=N)
    eperm = np.argsort(col, kind="stable")
    node_ptr = np.zeros(N + 1, np.int64)
    node_ptr[1:] = np.cumsum(deg)

    order = plan["order"]
    nk = plan["nk"]

    ea = np.asarray(edge_attr, dtype=np.float32)
    ub = np.asarray(u, dtype=np.float32)[np.asarray(batch, dtype=np.int64), 0]

    # per-core node permutation (sentinel N for padding)
    pis = np.full((ncores, nk), N, dtype=np.int64)
    coff = 0
    for d, m, s, cnt in plan["buckets"]:
        block = order[s:s + cnt]
        for k in range(ncores):
            mine = block[k::ncores]
            pis[k, coff:coff + len(mine)] = mine
        coff += m

    ea_streams, xT, urow = [], [], []
    for k in range(ncores):
        parts = []
        for d, m, s, cnt in plan["buckets"]:
            if d == 0:
                continue
            mine = order[s:s + cnt][k::ncores]
            ktot = -(-m // P)
            M = ktot * P
            idx = np.full((M, d), E, dtype=np.int64)
            if len(mine):
                idx[:len(mine)] = node_ptr[mine][:, None] + np.arange(d)[None, :]
                idx[:len(mine)] = eperm[idx[:len(mine)]]
            sent = idx == E
            blk = ea[np.where(sent, 0, idx)]
            if sent.any():
                blk[sent] = 0.0
            blk = blk.reshape(M, d, FEAT)
            ck = _chunk_cols(d)
            for c0 in range(0, ktot, ck):
                kc = min(ck, ktot - c0)
                sub = blk[c0 * P:(c0 + kc) * P]          # [kc*128, d, 48]
                sub = sub.reshape(kc, P, d, FEAT).transpose(1, 2, 0, 3)
                parts.append(np.ascontiguousarray(sub).ravel())
        if parts:
            flat = np.concatenate(parts).astype(BF16)
        else:
            flat = np.zeros(P, np.float32).astype(BF16)
        ea_streams.append(flat)

        pk = pis[k]
        sentn = pk == N
        pk_safe = np.where(sentn, 0, pk)
        xk = np.asarray(x, dtype=np.float32)[pk_safe]
        uk = ub[pk_safe].copy()
        if sentn.any():
            xk[sentn] = 0.0
            uk[sentn] = 0.0
        xT.append(np.ascontiguousarray(xk.T).astype(BF16))
        urow.append(np.ascontiguousarray(uk[None, :]).astype(BF16))

    return pis, ea_streams, xT, urow


# ------------------------------------------------------------- device program

def _build_program(buckets, nk, e_total, ncores):
    import concourse.bacc as bacc
    import concourse.mybir as mybir
    import concourse.tile as tile
    from concourse.masks import make_identity

    f32 = mybir.dt.float32
    bf16 = mybir.dt.bfloat16
    nc = bacc.Bacc("TRN2", target_bir_lowering=False, debug=False,
                   num_devices=ncores)

    ea = nc.dram_tensor("ea", [max(e_total, P)], bf16, kind="ExternalInput")
    xT = nc.dram_tensor("xT", [XF, nk], bf16, kind="ExternalInput")
    urow = nc.dram_tensor("urow", [1, nk], bf16, kind="ExternalInput")
    w1a_d = nc.dram_tensor("W1A", [XF + FEAT, HID], bf16, kind="ExternalInput")
    w1b_d = nc.dram_tensor("W1B", [NB, HID], bf16, kind="ExternalInput")
    w2_d = nc.dram_tensor("W2", [HID, XF], bf16, kind="ExternalInput")
    b1_d = nc.dram_tensor("b1", [HID, 1], f32, kind="ExternalInput")
    b2_d = nc.dram_tensor("b2", [XF, 1], f32, kind="ExternalInput")
    outT = nc.dram_tensor("outT", [XF, nk], bf16, kind="ExternalOutput")

    with tile.TileContext(nc) as tc:
        with tc.tile_pool(name="const", bufs=1) as cp, \
             tc.tile_pool(name="hp", bufs=1) as hp, \
             tc.tile_pool(name="edges", bufs=3) as ep, \
             tc.tile_pool(name="tree", bufs=1) as tp, \
             tc.tile_pool(name="red", bufs=3) as rp, \
             tc.tile_pool(name="mlp", bufs=2) as mp, \
             tc.tile_pool(name="pst", bufs=4, space="PSUM") as pst, \
             tc.tile_pool(name="psm", bufs=2, space="PSUM") as psm:

            ident = cp.tile([P, P], bf16)
            make_identity(nc, ident[:])
            w1a = cp.tile([XF + FEAT, HID], bf16)
            nc.sync.dma_start(out=w1a[:], in_=w1a_d[:, :])
            w1b = cp.tile([NB, HID], bf16)
            nc.sync.dma_start(out=w1b[:], in_=w1b_d[:, :])
            w2 = cp.tile([HID, XF], bf16)
            nc.sync.dma_start(out=w2[:], in_=w2_d[:, :])
            b1t = cp.tile([HID, 1], f32)
            nc.sync.dma_start(out=b1t[:], in_=b1_d[:, :])
            b2t = cp.tile([XF, 1], f32)
            nc.sync.dma_start(out=b2t[:], in_=b2_d[:, :])

            hA = hp.tile([XF + FEAT, nk], bf16)  # s(0:48) | pad | m(64:112)
            # prime every ps-pool slot: rows 48:64 are never written by the
            # transposes but are copied into hA's zero-weight pad rows, so
            # they must be finite (PSUM starts as junk, possibly NaN)
            zwide = cp.tile([P, 32], bf16)
            nc.vector.memset(zwide[:, :], 0.0)
            for _ in range(4):
                pst_prime = pst.tile([XF + FEAT, MAXW], bf16, tag="ts")
                for o in range(0, MAXW, P):
                    nc.tensor.transpose(out=pst_prime[32:XF, o:o + P],
                                        in_=zwide[:, :], identity=ident[:, :])
            hB = hp.tile([NB, nk], bf16)         # x(0:64) | mean(64:112) | u
            nc.sync.dma_start(out=hB[NB - 1:NB, :], in_=urow[:, :])

            # ---- fused node MLP, emitted per column chunk as soon as the
            # producing buckets have been emitted (lets Tile overlap the MLP
            # with later buckets' DMA/reduce work)
            def emit_mlp_chunks(qs):
                ws = [min(MAXW, nk - q) for q in qs]
                lo = qs[0]
                hi = qs[-1] + min(MAXW, nk - qs[-1])
                nc.sync.dma_start(out=hB[0:XF, lo:hi], in_=xT[:, lo:hi])
                pm1s, h2s, pm2s, ots = [], [], [], []
                for q0, w in zip(qs, ws):
                    pm1 = psm.tile([HID, MAXW], f32, tag="mm1")
                    nc.tensor.matmul(out=pm1[:, 0:w], lhsT=w1a[:],
                                     rhs=hA[:, q0:q0 + w],
                                     start=True, stop=False)
                    pm1s.append(pm1)
                for q0, w, pm1 in zip(qs, ws, pm1s):
                    nc.tensor.matmul(out=pm1[:, 0:w], lhsT=w1b[:],
                                     rhs=hB[:, q0:q0 + w], start=False, stop=True)
                for q0, w, pm1 in zip(qs, ws, pm1s):
                    h2 = mp.tile([HID, MAXW], bf16, tag="h2")
                    nc.scalar.activation(out=h2[:, 0:w], in_=pm1[:, 0:w],
                                         func=mybir.ActivationFunctionType.Relu,
                                         bias=b1t[:, 0:1])
                    h2s.append(h2)
                for q0, w, h2 in zip(qs, ws, h2s):
                    pm2 = psm.tile([XF, MAXW], f32, tag="mm2")
                    nc.tensor.matmul(out=pm2[:, 0:w], lhsT=w2[:],
                                     rhs=h2[:, 0:w], start=True, stop=False)
                    pm2s.append(pm2)
                for q0, w, pm2 in zip(qs, ws, pm2s):
                    nc.tensor.matmul(out=pm2[:, 0:w], lhsT=ident[0:XF, 0:XF],
                                     rhs=hB[0:XF, q0:q0 + w],
                                     start=False, stop=True)
                for q0, w, pm2 in zip(qs, ws, pm2s):
                    ot = mp.tile([XF, MAXW], bf16, tag="ot")
                    nc.scalar.activation(out=ot[:, 0:w], in_=pm2[:, 0:w],
                                         func=mybir.ActivationFunctionType.Identity,
                                         bias=b2t[:, 0:1])
                    ots.append(ot)
                for q0, w, ot in zip(qs, ws, ots):
                    nc.scalar.dma_start(out=outT[:, q0:q0 + w], in_=ot[:, 0:w])

            def emit_mlp_chunk(q0):
                emit_mlp_chunks([q0])

            # ---- pairwise reduction trees (Vector engine, bf16 2x mode)
            def emit_sum_tree(et, d, seg, st2):
                n, cur = d, et
                cA = cB = None
                use_a = True
                while n > 1:
                    h = n // 2
                    odd = n % 2
                    if h == 1 and not odd:
                        dst = st2
                    elif use_a:
                        if cA is None:
                            cA = tp.tile([P, (d // 2 + 1) * seg], bf16, tag="sA")
                        dst = cA
                    else:
                        if cB is None:
                            cB = tp.tile([P, (d // 4 + 2) * seg], bf16, tag="sB")
                        dst = cB
                    nc.vector.tensor_add(out=dst[:, 0:h * seg],
                                         in0=cur[:, 0:h * seg],
                                         in1=cur[:, h * seg:2 * h * seg])
                    if odd:
                        nc.scalar.copy(out=dst[:, h * seg:(h + 1) * seg],
                                       in_=cur[:, 2 * h * seg:(2 * h + 1) * seg])
                    n = h + odd
                    cur = dst
                    use_a = not use_a

            def emit_max_tree(et, d, seg, st2, moff):
                n, cur = d, et
                cA = cB = None
                use_a = True
                while n > 1:
                    h = (n + 1) // 2
                    if h == 1:
                        dst = st2[:, moff:moff + seg]
                    elif use_a:
                        if cA is None:
                            cA = tp.tile([P, ((d + 1) // 2) * seg], bf16, tag="mA")
                        dst = cA
                    else:
                        if cB is None:
                            cB = tp.tile([P, ((d + 3) // 4) * seg], bf16, tag="mB")
                        dst = cB
                    nc.vector.tensor_max(out=dst[:, 0:h * seg],
                                         in0=cur[:, 0:h * seg],
                                         in1=cur[:, (n - h) * seg:n * seg])
                    n = h
                    cur = dst
                    use_a = not use_a

            col_off = 0
            e_off = 0
            mlp_q0 = 0
            for d, m, _, _ in buckets:
                if d == 0:
                    nc.vector.memset(hA[:, col_off:col_off + m], 0.0)
                    nc.vector.memset(hB[XF:XF + FEAT, col_off:col_off + m], 0.0)
                    col_off += m
                    while mlp_q0 + MAXW <= col_off:
                        emit_mlp_chunk(mlp_q0)
                        mlp_q0 += MAXW
                    continue
                ktot = -(-m // P)
                ck = _chunk_cols(d)
                for c0 in range(0, ktot, ck):
                    kc = min(ck, ktot - c0)
                    seg = kc * FEAT
                    sz = P * d * seg
                    et = ep.tile([P, ck * d * FEAT], bf16, tag="e")
                    nc.sync.dma_start(
                        out=et[:, 0:d * seg],
                        in_=ea[e_off:e_off + sz].rearrange("(p x) -> p x", p=P))
                    e_off += sz
                    if d == 1:
                        stS = et
                        stM = et
                    else:
                        stS = rp.tile([P, ck * FEAT], bf16, tag="ss")
                        stM = rp.tile([P, ck * FEAT], bf16, tag="sm")
                        emit_sum_tree(et, d, seg, stS)
                        emit_max_tree(et, d, seg, stM, 0)
                    # transpose s and m per 128-node column into one 112-row
                    # PSUM tile (s rows 0:48, m rows 64:112 - legal matmul
                    # output bases), then one wide scalar copy per group
                    for g0 in range(0, kc, 4):
                        g1 = min(g0 + 4, kc)
                        valid = m - (c0 + g0) * P
                        if valid <= 0:
                            break
                        ps = pst.tile([XF + FEAT, MAXW], bf16, tag="ts")
                        for j in range(g0, g1):
                            if m - (c0 + j) * P <= 0:
                                break
                            o = (j - g0) * P
                            nc.tensor.transpose(
                                out=ps[0:FEAT, o:o + P],
                                in_=stS[:, j * FEAT:(j + 1) * FEAT],
                                identity=ident[:, :])
                            nc.tensor.transpose(
                                out=ps[XF:XF + FEAT, o:o + P],
                                in_=stM[:, j * FEAT:(j + 1) * FEAT],
                                identity=ident[:, :])
                        cov = min((g1 - g0) * P, valid)
                        dst0 = col_off + (c0 + g0) * P
                        nc.scalar.copy(out=hA[:, dst0:dst0 + cov],
                                       in_=ps[:, 0:cov])
                        nc.scalar.mul(out=hB[XF:XF + FEAT, dst0:dst0 + cov],
                                      in_=ps[0:FEAT, 0:cov], mul=1.0 / d)
                col_off += m
                ready = []
                while mlp_q0 + MAXW <= col_off:
                    ready.append(mlp_q0)
                    mlp_q0 += MAXW
                for i in range(0, len(ready), 2):
                    emit_mlp_chunks(ready[i:i + 2])

            while mlp_q0 < nk:
                emit_mlp_chunk(mlp_q0)
                mlp_q0 += MAXW

    nc.compile()
    return nc


# ----------------------------------------------------------------------- main

def kernel(**inputs):
    global LAST_EXEC_NS
    from concourse.bass_utils import run_bass_kernel_spmd

    x = np.asarray(inputs["x"], dtype=np.float32)
    edge_index = np.asarray(inputs["edge_index"])
    edge_attr = np.asarray(inputs["edge_attr"], dtype=np.float32)
    u = np.asarray(inputs["u"], dtype=np.float32)
    batch = np.asarray(inputs["batch"])
    W1 = np.asarray(inputs["W1"], dtype=np.float32)
    b1 = np.asarray(inputs["b1"], dtype=np.float32)
    W2 = np.asarray(inputs["W2"], dtype=np.float32)
    b2 = np.asarray(inputs["b2"], dtype=np.float32)

    N = x.shape[0]
    col = np.asarray(edge_index[1], dtype=np.int64)
    deg = np.bincount(col, minlength=N)
    plan = _make_plan(deg, NCORES)
    buckets = plan["buckets"]
    nk, e_total = plan["nk"], plan["e_total"]

    key = (N, edge_attr.shape[0], tuple((d, m) for d, m, _, _ in buckets))
    if key not in _PROG_CACHE:
        _PROG_CACHE[key] = _build_program(buckets, nk, e_total, NCORES)
    nc = _PROG_CACHE[key]

    pis, ea_s, xT_s, u_s = _host_pack(x, edge_index, edge_attr, u, batch,
                                      plan, NCORES)

    mlp_in = W1.shape[0]                     # 209
    w1a = np.zeros((XF + FEAT, HID), np.float32)
    w1a[0:FEAT] = W1[XF:XF + FEAT]                     # s rows
    w1a[XF:XF + FEAT] = W1[XF + FEAT:XF + 2 * FEAT]    # m rows
    w1b = np.zeros((NB, HID), np.float32)
    w1b[0:XF] = W1[0:XF]                               # x rows
    w1b[XF:XF + FEAT] = W1[XF + 2 * FEAT:XF + 3 * FEAT]  # mean rows
    w1b[XF + FEAT] = W1[mlp_in - 1]                    # u row
    in_maps = []
    for k in range(NCORES):
        in_maps.append({
            "ea": ea_s[k], "xT": xT_s[k], "urow": u_s[k],
            "W1A": np.ascontiguousarray(w1a).astype(BF16),
            "W1B": w1b.astype(BF16),
            "W2": np.ascontiguousarray(W2).astype(BF16),
            "b1": np.ascontiguousarray(b1.reshape(HID, 1)),
            "b2": np.ascontiguousarray(b2.reshape(XF, 1)),
        })

    trace = bool(int(os.environ.get("KERNEL_TRACE", "0")))
    kwargs = {}
    if trace:
        tdir = os.environ.get("KERNEL_TRACE_DIR") or None
        kwargs = {"trace": True, "tmpdir": tdir}
    res = run_bass_kernel_spmd(nc, in_maps, core_ids=list(range(NCORES)),
                               **kwargs)
    LAST_EXEC_NS = res.exec_time_ns

    out = np.empty((N, XF), np.float32)
    for k in range(NCORES):
        ok = res.results[k]["outT"].T.astype(np.float32)   # [nk, 64]
        pk = pis[k]
        valid = pk != N
        out[pk[valid]] = ok[valid]
    return out
